# revision 6
# baseline (speedup 1.0000x reference)
"""Trainium2 Bass kernel for the ComirecDR capsule-routing module.

Strategy (pure data parallel, per sharding hint):
  - shard batch B=4096 across 8 cores (512 rows each), replicate w.
  - fp16 feeds: item/w cast to fp16 on host; PE matmuls run fp16 with
    fp32 PSUM accumulation (4x the fp32 rate).
  - hat is materialized in SBUF twice, as hat1[b,i,e,s] and
    hat2[b,i,s,e] (both fp16, copied from PSUM by the Scalar engine),
    so every big DVE pass has unit-stride innermost operands and is
    eligible for the packed 2x mode:
      * cap contraction (sum over s) reads hat1, reduce over axis X
      * delta contraction (sum over e) reads hat2, reduce over axis X
  - routing iteration 0 is algebraic: sw0 = mask/50, so
    cap0 = (1/50) * (mask*item) @ w is a single K=3200 PE accumulation
    over host-premasked items (mitemT) -- no DVE pass at all.
  - squash is re-associated off the critical path: delta uses the
    UNSCALED capr (sum_s sw*hat before normalize/squash) and the scalar
    g = squash_factor * rs is applied to the small [b,I,S] delta
    afterwards; the sqrt/reciprocal chain overlaps the big DVE passes.
"""

import sys

sys.path.insert(0, "/opt/trn_rl_repo")

import numpy as np

import concourse.bass as bass
import concourse.bacc as bacc
import concourse.mybir as mybir
from concourse.tile import TileContext

B, S, I, E = 4096, 50, 4, 64
M = I * E  # 256
NCORES = 8
BSH = B // NCORES  # 512 batch rows per core
PT = 128  # batch rows per partition tile
NT = BSH // PT  # 4 tiles per core
KT = S * E // 128  # 25 K-tiles of 128 for the cap0 contraction
F32 = mybir.dt.float32
FP16 = mybir.dt.float16
AX = mybir.AxisListType
ACT = mybir.ActivationFunctionType
EPS = 1e-9
HALF = np.float16


def build_program():
    nc = bacc.Bacc("TRN2", target_bir_lowering=False, debug=False)
    itemT_d = nc.declare_dram_parameter("itemT", [E, NT, S, PT], FP16, isOutput=False)
    mitemT_d = nc.declare_dram_parameter(
        "mitemT", [128, NT, KT, PT], FP16, isOutput=False
    )
    wT_d = nc.declare_dram_parameter("wT", [E, S, M], FP16, isOutput=False)
    wT2_d = nc.declare_dram_parameter("wT2", [128, KT, M], FP16, isOutput=False)
    mf_d = nc.declare_dram_parameter("mf", [BSH, S], F32, isOutput=False)
    out_d = nc.declare_dram_parameter("out", [BSH, M], F32, isOutput=True)

    with TileContext(nc) as tc:
        with (
            tc.tile_pool(name="consts", bufs=1) as consts,
            tc.tile_pool(name="io", bufs=1) as io,
            tc.tile_pool(name="hats", bufs=2) as hats,
            tc.tile_pool(name="tmp", bufs=1) as tp,
            tc.tile_pool(name="small", bufs=2) as sm,
            tc.tile_pool(name="outp", bufs=2) as outp,
            tc.tile_pool(name="psum", bufs=2, space="PSUM") as pp,
            tc.tile_pool(name="psum_cap", bufs=2, space="PSUM") as cp,
            tc.tile_pool(name="psum_f", bufs=1, space="PSUM") as pf,
        ):
            wTc = consts.tile([E, S, M], FP16)
            nc.sync.dma_start(wTc[:], wT_d[:])
            wT2c = consts.tile([128, KT, M], FP16)
            nc.sync.dma_start(wT2c[:], wT2_d[:])

            # PE fences: LDWEIGHTS carries a single sync-wait, so throwaway
            # matmuls absorb the weight-DMA waits.
            fw = pf.tile([1, 1], F32, tag="fence")
            nc.tensor.matmul(fw[:], lhsT=wTc[:, 0, 0:1], rhs=wTc[:, 0, 0:1],
                             start=True, stop=True)
            fw2 = pf.tile([1, 1], F32, tag="fence")
            nc.tensor.matmul(fw2[:], lhsT=wT2c[:, 0, 0:1], rhs=wT2c[:, 0, 0:1],
                             start=True, stop=True)

            for t in range(NT):
                bsl = slice(t * PT, (t + 1) * PT)
                itemT = io.tile([E, S, PT], FP16, tag="itemT")
                nc.gpsimd.dma_start(itemT[:], itemT_d[:, t])
                mitemT = io.tile([128, KT, PT], FP16, tag="mitemT")
                nc.gpsimd.dma_start(mitemT[:], mitemT_d[:, t])
                mft = io.tile([PT, S], F32, tag="mf")
                nc.gpsimd.dma_start(mft[:], mf_d[bsl, :])

                f1 = pf.tile([1, 1], F32, tag="fence")
                nc.tensor.matmul(f1[:], lhsT=itemT[:, 0, 0:1],
                                 rhs=itemT[:, 0, 0:1], start=True, stop=True)
                f2 = pf.tile([1, 1], F32, tag="fence")
                nc.tensor.matmul(f2[:], lhsT=mitemT[:, 0, 0:1],
                                 rhs=mitemT[:, 0, 0:1], start=True, stop=True)

                # cap0_raw[b, (i,e)] = sum_s mf*hat = (mask*item) @ w
                cap0_ps = cp.tile([PT, M], F32, tag="cap0")
                for k in range(KT):
                    nc.tensor.matmul(
                        cap0_ps[:], lhsT=mitemT[:, k, :], rhs=wT2c[:, k, :],
                        start=(k == 0), stop=(k == KT - 1),
                    )

                # hat[b, (i,e)] per s; four s per 2-bank PSUM chunk, copied
                # to both SBUF layouts by the Scalar engine.
                hat1 = hats.tile([PT, I, E, S], FP16, tag="hat1")
                hat2 = hats.tile([PT, I, S, E], FP16, tag="hat2")
                for sp in range(0, S, 4):
                    cnt = min(4, S - sp)
                    ps = pp.tile([PT, 4, M], F32, tag="hatps")
                    for j in range(cnt):
                        s = sp + j
                        nc.tensor.matmul(
                            ps[:, j, :], lhsT=itemT[:, s, :], rhs=wTc[:, s, :],
                            start=True, stop=True,
                        )
                    src = ps[:, 0:cnt, :]
                    src1 = src.rearrange("p j (i e) -> p i e j", i=I, e=E)
                    nc.scalar.copy(hat1[:, :, :, sp : sp + cnt], src1)
                    src2 = src.rearrange("p j (i e) -> p i j e", i=I, e=E)
                    nc.scalar.copy(hat2[:, :, sp : sp + cnt, :], src2)

                tmp = tp.tile([PT, I * E * S], FP16, tag="tmp")
                tmp1v = tmp[:].rearrange("p (i e s) -> p i e s", i=I, e=E, s=S)
                tmp2v = tmp[:].rearrange("p (i s e) -> p i s e", i=I, s=S, e=E)

                cw = sm.tile([PT, I, S], F32, tag="cw")
                rs = None

                for it in range(3):
                    if it == 0:
                        # capr = cap0_raw from the PE, stored fp16
                        capr = sm.tile([PT, I, E], FP16, tag="capr")
                        nc.scalar.copy(
                            capr[:], cap0_ps[:].rearrange("p (i e) -> p i e", i=I)
                        )
                    else:
                        # masked softmax weights from cw
                        nmx = sm.tile([PT, I], F32, tag="nmx")
                        nc.vector.reduce_max(nmx, cw[:], axis=AX.X, negate=True)
                        ex = sm.tile([PT, I, S], F32, tag="ex")
                        for i in range(I):
                            nc.scalar.activation(
                                ex[:, i, :], cw[:, i, :], ACT.Exp,
                                bias=nmx[:, i : i + 1],
                            )
                        smm = sm.tile([PT, I], F32, tag="smm")
                        nc.vector.reduce_sum(smm, ex[:], axis=AX.X)
                        rs = sm.tile([PT, I], F32, tag="rs")
                        nc.vector.reciprocal(rs, smm)
                        exm_bf = sm.tile([PT, I, S], FP16, tag="exmbf")
                        nc.vector.tensor_mul(
                            exm_bf, ex[:],
                            mft[:, None, :].broadcast_to([PT, I, S]),
                        )
                        # capr[b,i,e] = sum_s exm*hat (unnormalized); DVE
                        # accumulates fp32 internally, single fp16 store
                        # rounding -- and the all-16-bit operands keep the
                        # packed 2x mode.
                        nc.vector.tensor_mul(
                            tmp1v, hat1[:],
                            exm_bf[:, :, None, :].broadcast_to([PT, I, E, S]),
                        )
                        capr = sm.tile([PT, I, E], FP16, tag="capr")
                        with nc.allow_low_precision("fp32 accum; fp16 store"):
                            nc.vector.reduce_sum(capr[:], tmp1v, axis=AX.X)

                    # squash factor on [PT, I]; v = rs*capr, n = |v|^2,
                    # g = rs * n/(1+n)/sqrt(n+eps); runs off the critical
                    # path (ACT norm + GpSimd chain + DVE reciprocals).
                    nraw = sm.tile([PT, I], F32, tag="nraw")
                    sq = sm.tile([PT, I, E], F32, tag="sq")
                    for i in range(I):
                        nc.scalar.activation(
                            sq[:, i, :], capr[:, i, :], ACT.Square,
                            accum_out=nraw[:, i : i + 1],
                        )
                    n_t = sm.tile([PT, I], F32, tag="n")
                    if it == 0:
                        nc.gpsimd.tensor_scalar_mul(n_t, nraw, 1.0 / (S * S))
                    else:
                        rs2 = sm.tile([PT, I], F32, tag="rs2")
                        nc.gpsimd.tensor_mul(rs2, rs, rs)
                        nc.gpsimd.tensor_mul(n_t, nraw, rs2)
                    a_t = sm.tile([PT, I], F32, tag="a")
                    nc.gpsimd.tensor_scalar_add(a_t, n_t, 1.0)
                    ra = sm.tile([PT, I], F32, tag="ra")
                    nc.vector.reciprocal(ra, a_t)
                    b_t = sm.tile([PT, I], F32, tag="b")
                    nc.gpsimd.tensor_scalar_add(b_t, n_t, EPS)
                    sb = sm.tile([PT, I], F32, tag="sb")
                    nc.scalar.activation(sb, b_t, ACT.Sqrt)
                    rb = sm.tile([PT, I], F32, tag="rb")
                    nc.vector.reciprocal(rb, sb)
                    g_t = sm.tile([PT, I], F32, tag="g")
                    nc.gpsimd.tensor_mul(g_t, n_t, ra)
                    nc.gpsimd.tensor_mul(g_t, g_t, rb)
                    if it == 0:
                        nc.gpsimd.tensor_scalar_mul(g_t, g_t, 1.0 / S)
                    else:
                        nc.gpsimd.tensor_mul(g_t, g_t, rs)

                    if it < 2:
                        # delta_raw[b,i,s] = sum_e hat*capr ; cw += g*delta_raw
                        nc.vector.tensor_mul(
                            tmp2v, hat2[:],
                            capr[:, :, None, :].broadcast_to([PT, I, S, E]),
                        )
                        draw = sm.tile([PT, I, S], FP16, tag="draw")
                        with nc.allow_low_precision("fp32 accum; fp16 store"):
                            nc.vector.reduce_sum(draw[:], tmp2v, axis=AX.X)
                        if it == 0:
                            nc.vector.tensor_mul(
                                cw[:], draw[:],
                                g_t[:, :, None].broadcast_to([PT, I, S]),
                            )
                        else:
                            gd = sm.tile([PT, I, S], F32, tag="gd")
                            nc.vector.tensor_mul(
                                gd[:], draw[:],
                                g_t[:, :, None].broadcast_to([PT, I, S]),
                            )
                            nc.vector.tensor_add(cw[:], cw[:], gd[:])
                    else:
                        outt = outp.tile([PT, M], F32, tag="outt")
                        nc.vector.tensor_mul(
                            outt[:].rearrange("p (i e) -> p i e", i=I),
                            capr[:],
                            g_t[:, :, None].broadcast_to([PT, I, E]),
                        )
                        nc.gpsimd.dma_start(out_d[bsl, :], outt[:])

    nc.compile()
    return nc


_runner = None


def _get_runner():
    """Build the bass program once and wrap it in a cached shard_map-jitted
    callable over the 8 NeuronCores (mirrors bass2jax.run_bass_via_pjrt)."""
    global _runner
    if _runner is not None:
        return _runner

    import jax
    from jax.experimental.shard_map import shard_map
    from jax.sharding import Mesh, PartitionSpec

    from concourse import bass2jax
    import concourse.mybir as _mybir

    nc = build_program()
    bass2jax.install_neuronx_cc_hook()

    partition_name = (
        nc.partition_id_tensor.name if nc.partition_id_tensor else None
    )
    in_names = []
    out_names = []
    out_avals = []
    for alloc in nc.m.functions[0].allocations:
        if not isinstance(alloc, _mybir.MemoryLocationSet):
            continue
        name = alloc.memorylocations[0].name
        if alloc.kind == "ExternalInput":
            if name != partition_name:
                in_names.append(name)
        elif alloc.kind == "ExternalOutput":
            out_names.append(name)
            out_avals.append(
                jax.core.ShapedArray(
                    tuple(alloc.tensor_shape), _mybir.dt.np(alloc.dtype)
                )
            )
    n_params = len(in_names)
    n_outs = len(out_avals)
    all_in_names = tuple(
        in_names + out_names + ([partition_name] if partition_name else [])
    )
    donate = tuple(range(n_params, n_params + n_outs))

    def _body(*args):
        operands = list(args)
        if partition_name is not None:
            operands.append(bass2jax.partition_id_tensor())
        outs = bass2jax._bass_exec_p.bind(
            *operands,
            out_avals=tuple(out_avals),
            in_names=all_in_names,
            out_names=tuple(out_names),
            lowering_input_output_aliases=(),
            sim_require_finite=True,
            sim_require_nnan=True,
            nc=nc,
        )
        return tuple(outs)

    devices = jax.devices()[:NCORES]
    mesh = Mesh(np.asarray(devices), ("core",))
    in_specs = (PartitionSpec("core"),) * (n_params + n_outs)
    out_specs = (PartitionSpec("core"),) * n_outs
    sharded = jax.jit(
        shard_map(
            _body, mesh=mesh, in_specs=in_specs, out_specs=out_specs,
            check_rep=False,
        ),
        donate_argnums=donate,
        keep_unused=True,
    )

    zero_out_shapes = [
        ((NCORES * a.shape[0],) + tuple(a.shape[1:]), a.dtype) for a in out_avals
    ]

    def runner(concat_inputs_by_name):
        concat_in = [concat_inputs_by_name[n] for n in in_names]
        concat_zeros = [np.zeros(s, d) for s, d in zero_out_shapes]
        out_arrs = sharded(*concat_in, *concat_zeros)
        return {n: out_arrs[i] for i, n in enumerate(out_names)}

    _runner = runner
    return _runner


def _prep_inputs(item_eb, mask, w):
    item = np.asarray(item_eb, dtype=np.float32)
    mask_np = np.asarray(mask)
    w0 = np.asarray(w, dtype=np.float32)[0]  # [S, M, E]
    mf = mask_np.astype(np.float32)  # [B, S]
    mitem = item * mf[:, :, None]  # [B, S, E]

    # itemT: [8*E, NT, S, PT] -- per core [E, NT, S, PT]
    itemT = (
        item.reshape(NCORES, NT, PT, S, E)
        .transpose(0, 4, 1, 3, 2)
        .astype(HALF)
        .reshape(NCORES * E, NT, S, PT)
    )
    # mitemT: [8*128, NT, KT, PT]; partition p = (s - 2k)*E + e
    mitemT = (
        mitem.reshape(NCORES, NT, PT, KT, 2, E)
        .transpose(0, 4, 5, 1, 3, 2)
        .astype(HALF)
        .reshape(NCORES * 128, NT, KT, PT)
    )
    # wT: [E, S, M] replicated; wT2: [128=(ds,e), KT, M] replicated
    wT = np.ascontiguousarray(w0.transpose(2, 0, 1)).astype(HALF)  # [E, S, M]
    wT2 = (
        w0.reshape(KT, 2, M, E).transpose(1, 3, 0, 2).astype(HALF).reshape(128, KT, M)
    )
    wT_cat = np.concatenate([wT] * NCORES, axis=0)
    wT2_cat = np.concatenate([wT2] * NCORES, axis=0)
    return {
        "itemT": np.ascontiguousarray(itemT),
        "mitemT": np.ascontiguousarray(mitemT),
        "wT": wT_cat,
        "wT2": wT2_cat,
        "mf": mf,
    }


def _run(item_eb, mask, w):
    runner = _get_runner()
    ins = _prep_inputs(item_eb, mask, w)
    outs = runner(ins)
    out = np.asarray(outs["out"])  # [8*BSH, M]
    return out.reshape(B, I, E)


def kernel(item_eb, mask, w):
    return _run(item_eb, mask, w)


# revision 7
# speedup vs baseline: 1.2309x; 1.2309x over previous
"""Trainium2 Bass kernel for the ComirecDR capsule-routing module.

Strategy (pure data parallel, per sharding hint):
  - shard batch B=4096 across 8 cores (512 rows each), replicate w.
  - fp16 feeds: item/w cast to fp16 on host; PE matmuls run fp16 with
    fp32 PSUM accumulation (4x the fp32 rate).
  - hat is materialized in SBUF twice, as hat1[b,i,e,s] and
    hat2[b,i,s,e] (both fp16, copied from PSUM by the Scalar engine),
    so every big DVE pass has unit-stride innermost operands and is
    eligible for the packed 2x mode:
      * cap contraction (sum over s) reads hat1, reduce over axis X
      * delta contraction (sum over e) reads hat2, reduce over axis X
  - routing iteration 0 is algebraic: sw0 = mask/50, so
    cap0 = (1/50) * (mask*item) @ w is a single K=3200 PE accumulation
    over host-premasked items (mitemT) -- no DVE pass at all.
  - squash is re-associated off the critical path: delta uses the
    UNSCALED capr (sum_s sw*hat before normalize/squash) and the scalar
    g = squash_factor * rs is applied to the small [b,I,S] delta
    afterwards; the sqrt/reciprocal chain overlaps the big DVE passes.
"""

import sys

sys.path.insert(0, "/opt/trn_rl_repo")

import numpy as np

import concourse.bass as bass
import concourse.bacc as bacc
import concourse.mybir as mybir
from concourse.tile import TileContext

B, S, I, E = 4096, 50, 4, 64
M = I * E  # 256
NCORES = 8
BSH = B // NCORES  # 512 batch rows per core
PT = 128  # batch rows per partition tile
NT = BSH // PT  # 4 tiles per core
KT = S * E // 128  # 25 K-tiles of 128 for the cap0 contraction
F32 = mybir.dt.float32
FP16 = mybir.dt.float16
AX = mybir.AxisListType
ACT = mybir.ActivationFunctionType
EPS = 1e-9
HALF = np.float16


def build_program():
    nc = bacc.Bacc("TRN2", target_bir_lowering=False, debug=False)
    itemT_d = nc.declare_dram_parameter("itemT", [E, NT, S, PT], FP16, isOutput=False)
    mitemT_d = nc.declare_dram_parameter(
        "mitemT", [128, NT, KT, PT], FP16, isOutput=False
    )
    wT_d = nc.declare_dram_parameter("wT", [E, S, M], FP16, isOutput=False)
    wT2_d = nc.declare_dram_parameter("wT2", [128, KT, M], FP16, isOutput=False)
    mf_d = nc.declare_dram_parameter("mf", [BSH, S], F32, isOutput=False)
    out_d = nc.declare_dram_parameter("out", [BSH, M], F32, isOutput=True)

    # s-axis pieces for the chunked weight/item DMAs: finer granularity lets
    # the first hat matmuls (and the Scalar-engine copies behind them) start
    # before the full transfer lands, shrinking the pipeline ramp.
    PIECES = [(0, 12), (12, 24), (24, 36), (36, 48), (48, 50)]

    def piece_of(s):
        return min(s // 12, 4)

    with TileContext(nc) as tc:
        with (
            tc.tile_pool(name="consts", bufs=1) as consts,
            tc.tile_pool(name="io", bufs=1) as io,
            tc.tile_pool(name="hats", bufs=2) as hats,
            tc.tile_pool(name="tmp", bufs=1) as tp,
            tc.tile_pool(name="small", bufs=2) as sm,
            tc.tile_pool(name="outp", bufs=2) as outp,
            tc.tile_pool(name="psum", bufs=2, space="PSUM") as pp,
            tc.tile_pool(name="psum_cap", bufs=2, space="PSUM") as cp,
            tc.tile_pool(name="psum_f", bufs=1, space="PSUM") as pf,
        ):
            wps = []
            for pi, (a, b) in enumerate(PIECES):
                w_p = consts.tile([E, b - a, M], FP16, tag=f"wT{pi}")
                nc.sync.dma_start(w_p[:], wT_d[:, a:b, :])
                wps.append(w_p)
            wT2c = consts.tile([128, KT, M], FP16)
            nc.sync.dma_start(wT2c[:], wT2_d[:])

            # PE fences: LDWEIGHTS carries a single sync-wait, so throwaway
            # matmuls absorb the weight-DMA waits.
            for pi in range(len(PIECES)):
                fw = pf.tile([1, 1], F32, tag="fence")
                nc.tensor.matmul(fw[:], lhsT=wps[pi][:, 0, 0:1],
                                 rhs=wps[pi][:, 0, 0:1], start=True, stop=True)
            fw2 = pf.tile([1, 1], F32, tag="fence")
            nc.tensor.matmul(fw2[:], lhsT=wT2c[:, 0, 0:1], rhs=wT2c[:, 0, 0:1],
                             start=True, stop=True)

            for t in range(NT):
                bsl = slice(t * PT, (t + 1) * PT)
                ips = []
                for pi, (a, b) in enumerate(PIECES):
                    i_p = io.tile([E, b - a, PT], FP16, tag=f"itemT{pi}")
                    nc.gpsimd.dma_start(i_p[:], itemT_d[:, t, a:b, :])
                    ips.append(i_p)
                mitemT = io.tile([128, KT, PT], FP16, tag="mitemT")
                nc.gpsimd.dma_start(mitemT[:], mitemT_d[:, t])
                mft = io.tile([PT, S], F32, tag="mf")
                nc.gpsimd.dma_start(mft[:], mf_d[bsl, :])

                for pi in range(len(PIECES)):
                    f1 = pf.tile([1, 1], F32, tag="fence")
                    nc.tensor.matmul(f1[:], lhsT=ips[pi][:, 0, 0:1],
                                     rhs=ips[pi][:, 0, 0:1], start=True,
                                     stop=True)
                f2 = pf.tile([1, 1], F32, tag="fence")
                nc.tensor.matmul(f2[:], lhsT=mitemT[:, 0, 0:1],
                                 rhs=mitemT[:, 0, 0:1], start=True, stop=True)

                # hat[b, (i,e)] per s; four s per 2-bank PSUM chunk, copied
                # to both SBUF layouts by the Scalar engine. The cap0
                # accumulation is interleaved after chunk 6 so its result is
                # ready when hat2[:, :, 0:25] lands (first delta0 half).
                hat1 = hats.tile([PT, I, E, S], FP16, tag="hat1")
                hat2 = hats.tile([PT, I, S, E], FP16, tag="hat2")
                cap0_ps = cp.tile([PT, M], F32, tag="cap0")

                def hat_chunk(sp):
                    cnt = min(4, S - sp)
                    ps = pp.tile([PT, 4, M], F32, tag="hatps")
                    for j in range(cnt):
                        s = sp + j
                        pi = piece_of(s)
                        nc.tensor.matmul(
                            ps[:, j, :], lhsT=ips[pi][:, s - PIECES[pi][0], :],
                            rhs=wps[pi][:, s - PIECES[pi][0], :],
                            start=True, stop=True,
                        )
                    src = ps[:, 0:cnt, :]
                    src1 = src.rearrange("p j (i e) -> p i e j", i=I, e=E)
                    nc.scalar.copy(hat1[:, :, :, sp : sp + cnt], src1)
                    src2 = src.rearrange("p j (i e) -> p i j e", i=I, e=E)
                    nc.scalar.copy(hat2[:, :, sp : sp + cnt, :], src2)

                for sp in range(0, 28, 4):
                    hat_chunk(sp)
                for k in range(KT):
                    nc.tensor.matmul(
                        cap0_ps[:], lhsT=mitemT[:, k, :], rhs=wT2c[:, k, :],
                        start=(k == 0), stop=(k == KT - 1),
                    )
                for sp in range(28, S, 4):
                    hat_chunk(sp)

                tmp = tp.tile([PT, I * E * S], FP16, tag="tmp")
                tmp1v = tmp[:].rearrange("p (i e s) -> p i e s", i=I, e=E, s=S)
                tmp2v = tmp[:].rearrange("p (i s e) -> p i s e", i=I, s=S, e=E)

                cw = sm.tile([PT, I, S], F32, tag="cw")
                rs = None

                def cap_tree(dst_f32):
                    """sum over s of tmp1v[p,i,e,s]: two in-place fp16 tree
                    levels (packed 2x adds), then an exact fp32 tail reduce
                    over the remaining 13 columns."""
                    nc.vector.tensor_add(
                        tmp1v[:, :, :, 0:25], tmp1v[:, :, :, 0:25],
                        tmp1v[:, :, :, 25:50],
                    )
                    nc.vector.tensor_add(
                        tmp1v[:, :, :, 0:12], tmp1v[:, :, :, 0:12],
                        tmp1v[:, :, :, 13:25],
                    )
                    nc.vector.reduce_sum(dst_f32, tmp1v[:, :, :, 0:13], axis=AX.X)

                def delta_half(capr16, dst_f32, h0, h1):
                    """dst[p,i,h0:h1] = sum_e hat2[p,i,s,e]*capr16[p,i,e] for
                    the s-range [h0,h1): runs as soon as those hat2 rows and
                    capr16 are ready."""
                    tv = tmp2v[:, :, h0:h1, :]
                    nc.vector.tensor_mul(
                        tv, hat2[:, :, h0:h1, :],
                        capr16[:, :, None, :].broadcast_to(
                            [PT, I, h1 - h0, E]
                        ),
                    )
                    nc.vector.tensor_add(
                        tv[:, :, :, 0:32], tv[:, :, :, 0:32], tv[:, :, :, 32:64]
                    )
                    nc.vector.tensor_add(
                        tv[:, :, :, 0:16], tv[:, :, :, 0:16], tv[:, :, :, 16:32]
                    )
                    nc.vector.reduce_sum(dst_f32, tv[:, :, :, 0:16], axis=AX.X)

                for it in range(3):
                    if it == 0:
                        capr = sm.tile([PT, I, E], F32, tag="capr")
                        nc.scalar.copy(
                            capr[:], cap0_ps[:].rearrange("p (i e) -> p i e", i=I)
                        )
                    else:
                        # masked softmax weights from cw
                        nmx = sm.tile([PT, I], F32, tag="nmx")
                        nc.vector.reduce_max(nmx, cw[:], axis=AX.X, negate=True)
                        ex = sm.tile([PT, I, S], F32, tag="ex")
                        for i in range(I):
                            nc.scalar.activation(
                                ex[:, i, :], cw[:, i, :], ACT.Exp,
                                bias=nmx[:, i : i + 1],
                            )
                        smm = sm.tile([PT, I], F32, tag="smm")
                        nc.vector.reduce_sum(smm, ex[:], axis=AX.X)
                        rs = sm.tile([PT, I], F32, tag="rs")
                        nc.vector.reciprocal(rs, smm)
                        exm_bf = sm.tile([PT, I, S], FP16, tag="exmbf")
                        nc.vector.tensor_mul(
                            exm_bf, ex[:],
                            mft[:, None, :].broadcast_to([PT, I, S]),
                        )
                        # capr[b,i,e] = sum_s exm*hat (unnormalized)
                        nc.vector.tensor_mul(
                            tmp1v, hat1[:],
                            exm_bf[:, :, None, :].broadcast_to([PT, I, E, S]),
                        )
                        capr = sm.tile([PT, I, E], F32, tag="capr")
                        cap_tree(capr[:])

                    capr16 = sm.tile([PT, I, E], FP16, tag="capr16")
                    nc.vector.tensor_copy(capr16[:], capr[:])

                    # squash factor on [PT, I]; v = rs*capr, n = |v|^2,
                    # g = rs * n/(1+n)/sqrt(n+eps); off the critical path.
                    sq = sm.tile([PT, I, E], F32, tag="sq")
                    nc.vector.tensor_mul(sq[:], capr[:], capr[:])
                    nraw = sm.tile([PT, I], F32, tag="nraw")
                    nc.vector.reduce_sum(nraw, sq[:], axis=AX.X)
                    n_t = sm.tile([PT, I], F32, tag="n")
                    if it == 0:
                        nc.vector.tensor_scalar_mul(n_t, nraw, 1.0 / (S * S))
                    else:
                        rs2 = sm.tile([PT, I], F32, tag="rs2")
                        nc.vector.tensor_mul(rs2, rs, rs)
                        nc.vector.tensor_mul(n_t, nraw, rs2)
                    a_t = sm.tile([PT, I], F32, tag="a")
                    nc.vector.tensor_scalar_add(a_t, n_t, 1.0)
                    ra = sm.tile([PT, I], F32, tag="ra")
                    nc.vector.reciprocal(ra, a_t)
                    b_t = sm.tile([PT, I], F32, tag="b")
                    nc.vector.tensor_scalar_add(b_t, n_t, EPS)
                    sb = sm.tile([PT, I], F32, tag="sb")
                    nc.scalar.activation(sb, b_t, ACT.Sqrt)
                    rb = sm.tile([PT, I], F32, tag="rb")
                    nc.vector.reciprocal(rb, sb)
                    g_t = sm.tile([PT, I], F32, tag="g")
                    nc.vector.tensor_mul(g_t, n_t, ra)
                    nc.vector.tensor_mul(g_t, g_t, rb)
                    if it == 0:
                        nc.vector.tensor_scalar_mul(g_t, g_t, 1.0 / S)
                    else:
                        nc.vector.tensor_mul(g_t, g_t, rs)

                    if it < 2:
                        # delta_raw[b,i,s] = sum_e hat*capr ; cw += g*delta
                        draw = sm.tile([PT, I, S], F32, tag="draw")
                        delta_half(capr16, draw[:, :, 0:25], 0, 25)
                        delta_half(capr16, draw[:, :, 25:50], 25, 50)
                        if it == 0:
                            nc.vector.tensor_mul(
                                cw[:], draw[:],
                                g_t[:, :, None].broadcast_to([PT, I, S]),
                            )
                        else:
                            gd = sm.tile([PT, I, S], F32, tag="gd")
                            nc.vector.tensor_mul(
                                gd[:], draw[:],
                                g_t[:, :, None].broadcast_to([PT, I, S]),
                            )
                            nc.vector.tensor_add(cw[:], cw[:], gd[:])
                    else:
                        outt = outp.tile([PT, M], F32, tag="outt")
                        nc.vector.tensor_mul(
                            outt[:].rearrange("p (i e) -> p i e", i=I),
                            capr[:],
                            g_t[:, :, None].broadcast_to([PT, I, E]),
                        )
                        nc.gpsimd.dma_start(out_d[bsl, :], outt[:])

    nc.compile()
    return nc


_runner = None


def _get_runner():
    """Build the bass program once and wrap it in a cached shard_map-jitted
    callable over the 8 NeuronCores (mirrors bass2jax.run_bass_via_pjrt)."""
    global _runner
    if _runner is not None:
        return _runner

    import jax
    from jax.experimental.shard_map import shard_map
    from jax.sharding import Mesh, PartitionSpec

    from concourse import bass2jax
    import concourse.mybir as _mybir

    nc = build_program()
    bass2jax.install_neuronx_cc_hook()

    partition_name = (
        nc.partition_id_tensor.name if nc.partition_id_tensor else None
    )
    in_names = []
    out_names = []
    out_avals = []
    for alloc in nc.m.functions[0].allocations:
        if not isinstance(alloc, _mybir.MemoryLocationSet):
            continue
        name = alloc.memorylocations[0].name
        if alloc.kind == "ExternalInput":
            if name != partition_name:
                in_names.append(name)
        elif alloc.kind == "ExternalOutput":
            out_names.append(name)
            out_avals.append(
                jax.core.ShapedArray(
                    tuple(alloc.tensor_shape), _mybir.dt.np(alloc.dtype)
                )
            )
    n_params = len(in_names)
    n_outs = len(out_avals)
    all_in_names = tuple(
        in_names + out_names + ([partition_name] if partition_name else [])
    )
    donate = tuple(range(n_params, n_params + n_outs))

    def _body(*args):
        operands = list(args)
        if partition_name is not None:
            operands.append(bass2jax.partition_id_tensor())
        outs = bass2jax._bass_exec_p.bind(
            *operands,
            out_avals=tuple(out_avals),
            in_names=all_in_names,
            out_names=tuple(out_names),
            lowering_input_output_aliases=(),
            sim_require_finite=True,
            sim_require_nnan=True,
            nc=nc,
        )
        return tuple(outs)

    devices = jax.devices()[:NCORES]
    mesh = Mesh(np.asarray(devices), ("core",))
    in_specs = (PartitionSpec("core"),) * (n_params + n_outs)
    out_specs = (PartitionSpec("core"),) * n_outs
    sharded = jax.jit(
        shard_map(
            _body, mesh=mesh, in_specs=in_specs, out_specs=out_specs,
            check_rep=False,
        ),
        donate_argnums=donate,
        keep_unused=True,
    )

    zero_out_shapes = [
        ((NCORES * a.shape[0],) + tuple(a.shape[1:]), a.dtype) for a in out_avals
    ]

    def runner(concat_inputs_by_name):
        concat_in = [concat_inputs_by_name[n] for n in in_names]
        concat_zeros = [np.zeros(s, d) for s, d in zero_out_shapes]
        out_arrs = sharded(*concat_in, *concat_zeros)
        return {n: out_arrs[i] for i, n in enumerate(out_names)}

    _runner = runner
    return _runner


def _prep_inputs(item_eb, mask, w):
    item = np.asarray(item_eb, dtype=np.float32)
    mask_np = np.asarray(mask)
    w0 = np.asarray(w, dtype=np.float32)[0]  # [S, M, E]
    mf = mask_np.astype(np.float32)  # [B, S]
    mitem = item * mf[:, :, None]  # [B, S, E]

    # itemT: [8*E, NT, S, PT] -- per core [E, NT, S, PT]
    itemT = (
        item.reshape(NCORES, NT, PT, S, E)
        .transpose(0, 4, 1, 3, 2)
        .astype(HALF)
        .reshape(NCORES * E, NT, S, PT)
    )
    # mitemT: [8*128, NT, KT, PT]; partition p = (s - 2k)*E + e
    mitemT = (
        mitem.reshape(NCORES, NT, PT, KT, 2, E)
        .transpose(0, 4, 5, 1, 3, 2)
        .astype(HALF)
        .reshape(NCORES * 128, NT, KT, PT)
    )
    # wT: [E, S, M] replicated; wT2: [128=(ds,e), KT, M] replicated
    wT = np.ascontiguousarray(w0.transpose(2, 0, 1)).astype(HALF)  # [E, S, M]
    wT2 = (
        w0.reshape(KT, 2, M, E).transpose(1, 3, 0, 2).astype(HALF).reshape(128, KT, M)
    )
    wT_cat = np.concatenate([wT] * NCORES, axis=0)
    wT2_cat = np.concatenate([wT2] * NCORES, axis=0)
    return {
        "itemT": np.ascontiguousarray(itemT),
        "mitemT": np.ascontiguousarray(mitemT),
        "wT": wT_cat,
        "wT2": wT2_cat,
        "mf": mf,
    }


def _run(item_eb, mask, w):
    runner = _get_runner()
    ins = _prep_inputs(item_eb, mask, w)
    outs = runner(ins)
    out = np.asarray(outs["out"])  # [8*BSH, M]
    return out.reshape(B, I, E)


def kernel(item_eb, mask, w):
    return _run(item_eb, mask, w)
